# revision 1
# baseline (speedup 1.0000x reference)
"""LDPC belief-propagation kernel for Trainium2 (8 NeuronCores, data-parallel).

Math (per batch row, H fixed [3,7], 12 edges, check-major edge order):
  lu_e  = ln|tanh(m_e/2)|           = ln(1-z) - ln(1+z),  z = exp(-|m_e|)
  S_c   = sum_{e in check c} lu_e
  d_e   = S_c - lu_e                (== s_upd, <= 0)
  mag_e = -ln tanh(|d_e|/2)         = ln(1+u) - ln(1-u),  u = exp(d_e)
  sgn_e = prod_{e' in c} sign(m_{e'}) * sign(m_e)    (leave-one-out, +-1)
  c2v_e = mag_e * sgn_e
  new_llr_v = llr_v + sum_{c contains v} c2v_{c,v}
  m'_e  = new_llr_v - c2v_e
Only Exp/Ln/Abs/Sign activations -> one ACT table set, no table switches.
Edges of degree-1 variables (e0,e4,e8) carry constant messages == llr: their
lu/sign are computed once; per-iteration transcendentals cover only the 9
dynamic edges, and deg-1 new_llr terms are added only on the last iteration.
Batch is split into chunks so ACT/DVE/GPSIMD/DMA pipeline across chunks.
"""

import numpy as np

_CACHE = {}

NCORES = 8
P = 128      # partitions
CHUNKS = 2   # batch sub-chunks per core (pipeline depth)


def _build(Bc, iters):
    import contextlib

    import concourse.bass as bass
    import concourse.tile as tile
    from concourse import mybir
    from concourse.alu_op_type import AluOpType as Op

    F = mybir.ActivationFunctionType
    W = Bc // P // CHUNKS  # free columns per partition per chunk
    f32 = mybir.dt.float32

    nc = bass.Bass("TRN2", target_bir_lowering=False, debug=False,
                   num_devices=1)
    llr_d = nc.dram_tensor("llr", [Bc, 7], f32, kind="ExternalInput")
    out_d = nc.dram_tensor("out", [Bc, 7], f32, kind="ExternalOutput")

    def sub(t, off, dims):
        a = t[:] if callable(getattr(t, "__getitem__", None)) else t
        return bass.AP(tensor=a.tensor, offset=a.offset + off,
                       ap=[list(a.ap[0])] + [list(d) for d in dims])

    with tile.TileContext(nc) as tc:
        ctx = contextlib.ExitStack()
        with ctx:
            keep = ctx.enter_context(tc.tile_pool(name="keep", bufs=1))
            work = ctx.enter_context(tc.tile_pool(name="work", bufs=2))

            def K(name, k):
                return keep.tile([P, W * k], f32, tag=name, name=name)

            CB = keep.tile([P, 1], f32, tag="CB", name="CB")
            nc.vector.memset(CB, 1e-38)
            CB2 = keep.tile([P, 1], f32, tag="CB2", name="CB2")
            nc.vector.memset(CB2, 0.99999994)

            # per-chunk persistent state
            LLRs = [K(f"LLR{c}", 7) for c in range(CHUNKS)]
            Ms   = [K(f"M{c}", 12) for c in range(CHUNKS)]
            LUs  = [K(f"LU{c}", 12) for c in range(CHUNKS)]
            SGs  = [K(f"SG{c}", 12) for c in range(CHUNKS)]
            NLs  = [K(f"NL{c}", 7) for c in range(CHUNKS)]

            act = nc.scalar.activation
            vec = nc.vector
            gps = nc.gpsimd

            def g12(t):
                return sub(t, 0, [[12, W], [4, 3], [1, 4]])

            def dyn9(t):
                return sub(t, 1, [[12, W], [4, 3], [1, 3]])

            llr_ap = llr_d.ap().rearrange("(c p w) v -> c p (w v)", c=CHUNKS, p=P)
            out_ap = out_d.ap().rearrange("(c p w) v -> c p (w v)", c=CHUNKS, p=P)

            for c in range(CHUNKS):
                LLR, M = LLRs[c], Ms[c]
                nc.sync.dma_start(out=LLR[:], in_=llr_ap[c])
                vec.tensor_copy(sub(M, 0, [[12, W], [1, 4]]),
                                sub(LLR, 0, [[7, W], [2, 4]]))
                vec.tensor_copy(sub(M, 4, [[12, W], [1, 2]]),
                                sub(LLR, 1, [[7, W], [1, 2]]))
                vec.tensor_copy(sub(M, 6, [[12, W], [1, 2]]),
                                sub(LLR, 5, [[7, W], [1, 2]]))
                vec.tensor_copy(sub(M, 8, [[12, W], [1, 4]]),
                                sub(LLR, 3, [[7, W], [1, 4]]))

            for it in range(iters):
                full = (it == 0)
                lastit = (it == iters - 1)
                for c in range(CHUNKS):
                    LLR, M, LU, SG, NL = LLRs[c], Ms[c], LUs[c], SGs[c], NLs[c]
                    # scratch (tag-shared slots rotate across chunk bodies)
                    ZU  = work.tile([P, W * 12], f32, tag="ZU", name="ZU")
                    LPR = work.tile([P, W * 12], f32, tag="LPR", name="LPR")
                    LQS = work.tile([P, W * 12], f32, tag="LQS", name="LQS")
                    T6  = work.tile([P, W * 6], f32, tag="T6", name="T6")
                    S3  = work.tile([P, W * 3], f32, tag="S3", name="S3")
                    G6  = work.tile([P, W * 6], f32, tag="G6", name="G6")
                    G3  = work.tile([P, W * 3], f32, tag="G3", name="G3")
                    DM  = work.tile([P, W * 12], f32, tag="DM", name="DM")
                    SL  = work.tile([P, W * 12], f32, tag="SL", name="SL")
                    CV  = work.tile([P, W * 12], f32, tag="CV", name="CV")
                    TP  = work.tile([P, W * 2], f32, tag="TP", name="TP")

                    sl = (lambda t: t[:]) if full else dyn9
                    # phi1: lu = ln(1-z) - ln(1+z), z = exp(-|m|) clamped < 1
                    act(sl(ZU), sl(M), F.Abs)
                    act(sl(ZU), sl(ZU), F.Exp, scale=-1.0)
                    act(sl(LPR), sl(ZU), F.Ln, bias=1.0)
                    # scale/bias chosen so the argument stays >= 6e-8 even at
                    # z == 1.0 (m == +-0): keeps lu finite and strictly < 0
                    act(sl(LQS), sl(ZU), F.Ln, bias=CB2[:], scale=-0.99999988)
                    vec.tensor_tensor(sl(LU), sl(LQS), sl(LPR), Op.subtract)
                    # sign (+1 at exact zero via tiny bias)
                    act(sl(SG), sl(M), F.Sign, bias=CB[:])

                    # check sums / sign products
                    vec.tensor_tensor(T6[:], sub(LU, 0, [[12, W], [4, 3], [1, 2]]),
                                      sub(LU, 2, [[12, W], [4, 3], [1, 2]]), Op.add)
                    vec.tensor_tensor(S3[:], sub(T6, 0, [[6, W], [2, 3]]),
                                      sub(T6, 1, [[6, W], [2, 3]]), Op.add)
                    gps.tensor_tensor(G6[:], sub(SG, 0, [[12, W], [4, 3], [1, 2]]),
                                      sub(SG, 2, [[12, W], [4, 3], [1, 2]]), Op.mult)
                    gps.tensor_tensor(G3[:], sub(G6, 0, [[6, W], [2, 3]]),
                                      sub(G6, 1, [[6, W], [2, 3]]), Op.mult)

                    slg = g12 if lastit else dyn9
                    slf = (lambda t: t[:]) if lastit else dyn9
                    S3r = sub(S3, 0, [[3, W], [1, 3], [0, 4 if lastit else 3]])
                    G3r = sub(G3, 0, [[3, W], [1, 3], [0, 4 if lastit else 3]])
                    vec.tensor_tensor(slg(DM), S3r, slg(LU), Op.subtract)
                    act(slf(ZU), slf(DM), F.Exp)
                    act(slf(LPR), slf(ZU), F.Ln, bias=1.0)
                    act(slf(LQS), slf(ZU), F.Ln, bias=1.0, scale=-1.0)
                    gps.tensor_tensor(slg(SL), G3r, slg(SG), Op.mult)
                    vec.tensor_tensor(slf(DM), slf(LPR), slf(LQS), Op.subtract)
                    vec.tensor_tensor(slf(CV), slf(DM), slf(SL), Op.mult)

                    # new_llr for feedback vars v2,v5 (pairs), v4, v6
                    vec.tensor_tensor(TP[:], sub(CV, 1, [[12, W], [5, 2]]),
                                      sub(CV, 5, [[12, W], [5, 2]]), Op.add)
                    vec.tensor_tensor(sub(NL, 2, [[7, W], [3, 2]]),
                                      sub(LLR, 2, [[7, W], [3, 2]]),
                                      TP[:], Op.add)
                    vec.tensor_tensor(sub(NL, 4, [[7, W], [2, 2]]),
                                      sub(LLR, 4, [[7, W], [2, 2]]),
                                      sub(CV, 2, [[12, W], [1, 2]]), Op.add)
                    vec.tensor_tensor(sub(NL, 4, [[7, W], [2, 2]]),
                                      sub(NL, 4, [[7, W], [2, 2]]),
                                      sub(CV, 9, [[12, W], [-2, 2]]), Op.add)
                    vec.tensor_tensor(sub(NL, 6, [[7, W], [1, 1]]),
                                      sub(NL, 6, [[7, W], [1, 1]]),
                                      sub(CV, 11, [[12, W], [1, 1]]), Op.add)

                    if lastit:
                        vec.tensor_tensor(sub(NL, 0, [[7, W], [1, 2]]),
                                          sub(LLR, 0, [[7, W], [1, 2]]),
                                          sub(CV, 0, [[12, W], [4, 2]]), Op.add)
                        vec.tensor_tensor(sub(NL, 3, [[7, W], [1, 1]]),
                                          sub(LLR, 3, [[7, W], [1, 1]]),
                                          sub(CV, 8, [[12, W], [1, 1]]), Op.add)
                        nc.sync.dma_start(out=out_ap[c], in_=NL[:])
                    else:
                        # m' = new_llr - c2v for the 9 dynamic edges
                        vec.tensor_tensor(sub(M, 1, [[12, W], [1, 3]]),
                                          sub(NL, 2, [[7, W], [2, 3]]),
                                          sub(CV, 1, [[12, W], [1, 3]]), Op.subtract)
                        vec.tensor_tensor(sub(M, 9, [[12, W], [1, 3]]),
                                          sub(NL, 4, [[7, W], [1, 3]]),
                                          sub(CV, 9, [[12, W], [1, 3]]), Op.subtract)
                        vec.tensor_tensor(sub(M, 5, [[12, W], [1, 1]]),
                                          sub(NL, 2, [[7, W], [1, 1]]),
                                          sub(CV, 5, [[12, W], [1, 1]]), Op.subtract)
                        vec.tensor_tensor(sub(M, 6, [[12, W], [1, 2]]),
                                          sub(NL, 5, [[7, W], [1, 2]]),
                                          sub(CV, 6, [[12, W], [1, 2]]), Op.subtract)

    # walrus on this stack supports a single sync-wait slot per instruction.
    # Tile emits (a) redundant same-engine waits (trivially satisfied by the
    # engine's FIFO program order once the preceding updates have happened)
    # and (b) a kernel-tail SP drain waiting on the whole global clock, where
    # only the output-DMA wait is load-bearing (the per-engine drain + EVSEM
    # butterfly that follows enforces engine completion).  Strip both.
    import bass_rust
    pref = {"EngineType.DVE": "DVE_", "EngineType.Pool": "Pool_",
            "EngineType.Activation": "Activation_", "EngineType.PE": "PE_",
            "EngineType.SP": "SP_"}
    inc = {}
    for b in nc.m.functions[0].blocks:
        for i in b.instructions:
            si = i.sync_info
            if si is None:
                continue
            if len(si.on_wait) > 1:
                if type(i).__name__ == "InstDrain":
                    dma = [w for w in si.on_wait if "DMA" in w.ant_name]
                    keep_w = dma[-1:] if dma else list(si.on_wait)[:1]
                else:
                    p = pref.get(str(i.engine))
                    keep_w = [w for w in si.on_wait
                              if not (p and w.ant_name.startswith(p)
                                      and w.wait_value <= inc.get(w.ant_name, 0))]
                    assert len(keep_w) <= 1, (i.name, [(w.ant_name, w.wait_value) for w in keep_w], {k: inc.get(k) for k in [w.ant_name for w in si.on_wait]})
                i.sync_info = bass_rust.SyncInfo(on_wait=keep_w,
                                                on_update=list(si.on_update))
                si = i.sync_info
            for u in si.on_update:
                if u.update_mode == "sem-inc":
                    inc[u.ant_name] = inc.get(u.ant_name, 0) + u.update_value
    return nc


def kernel(llr, max_iters):
    llr = np.ascontiguousarray(np.asarray(llr), dtype=np.float32)
    iters = int(np.asarray(max_iters))
    B = llr.shape[0]
    if iters <= 0:
        return llr.reshape(B, 1, 7).copy()

    from concourse.bass_utils import run_bass_kernel_spmd

    Bc = B // NCORES
    key = (Bc, iters)
    if key not in _CACHE:
        _CACHE[key] = _build(Bc, iters)
    nc = _CACHE[key]

    flat = llr.reshape(B, 7)
    in_maps = [{"llr": flat[i * Bc:(i + 1) * Bc]} for i in range(NCORES)]
    res = run_bass_kernel_spmd(nc, in_maps, core_ids=list(range(NCORES)))
    out = np.concatenate([np.asarray(r["out"]) for r in res.results], axis=0)
    return out.reshape(B, 1, 7)



# revision 8
# speedup vs baseline: 1.3790x; 1.3790x over previous
"""LDPC belief-propagation kernel for Trainium2 (8 NeuronCores, data-parallel).

Tanh-product formulation (per row; H fixed [3,7], 12 edges, check-major
slots with each check's degree-1 "static" edge in slot 0):
  t_e   = tanh(m_e / 2)                       (signed; ACT Tanh)
  u_e   = prod_{e' in check(e), e'!=e} t_e'   (leave-one-out via pair trick)
  c2v_e = ln(1+u_e) - ln(1-u_e)               (= 2 artanh(u); sign comes free)
  new_llr_v = llr_v + sum_{c ni v} c2v_{c,v}
  m'_e  = new_llr_v - c2v_e
This needs only 3 ACT ops/iter (Tanh + 2 Ln) vs 8 for the log-domain form.
Leave-one-out uses pair products: P(pair) = t_a*t_b, then
u_e = t_partner(e) * P(other pair).  Degree-1 variables (v0,v1,v3) have
constant messages == llr: their t values are computed once ("static" slots
0,4,8); per-iteration work covers only the 9 dynamic edges.  Iteration 0
messages equal llr_v, so its tanh is folded into the one-time setup.

Engine split: all transcendentals on ACT; the c2v/new_llr/m' post-path on
DVE (fp16 mid-iteration for the 2x_1p packed mode, fp32 on the last
iteration for the exact output path).  The t-products run on GPSIMD/Pool
for chunk 0 and on DVE for chunk 1 — Pool's 0.42 mult efficiency makes the
optimal batch split uneven, and dedicating one product engine per chunk
keeps every instruction dependent on at most one foreign engine (the
sync-strip pass below requires a single wait slot per instruction).

Slot layout (check-major):
  c0: [v0*, v2, v4, v6]  slots 0-3
  c1: [v1*, v2, v5, v6]  slots 4-7
  c2: [v3*, v4, v5, v6]  slots 8-11   (* = static, degree-1)
"""

import numpy as np

_CACHE = {}

NCORES = 8
P = 128          # partitions
WS = (196, 60)   # free columns per partition per chunk (sum = Bc // P)

CA = 0.99999988  # Ln scale so the argument stays >= 6e-8 even at u == +-1
CB = 0.99999994  # keeps c2v finite and |c2v| <= ~16.8 (matches ref clamp)


def _build(Bc, iters):
    import contextlib

    import concourse.bass as bass
    import concourse.tile as tile
    from concourse import mybir
    from concourse.alu_op_type import AluOpType as Op

    F = mybir.ActivationFunctionType
    assert Bc == P * sum(WS), (Bc, WS)
    f32 = mybir.dt.float32
    f16 = mybir.dt.float16

    nc = bass.Bass("TRN2", target_bir_lowering=False, debug=False,
                   num_devices=1)
    llr_d = nc.dram_tensor("llr", [Bc, 7], f32, kind="ExternalInput")
    out_d = nc.dram_tensor("out", [Bc, 7], f32, kind="ExternalOutput")

    def sub(t, off, dims):
        a = t[:] if callable(getattr(t, "__getitem__", None)) else t
        return bass.AP(tensor=a.tensor, offset=a.offset + off,
                       ap=[list(a.ap[0])] + [list(d) for d in dims])

    def hbm_ap(t, row0, w):
        # [P, 7w] view of rows [row0, row0 + P*w): partition p <-> w rows
        a = t.ap()
        return bass.AP(tensor=a.tensor, offset=a.offset + 7 * row0,
                       ap=[[7 * w, P], [1, 7 * w]])

    with tile.TileContext(nc) as tc:
        ctx = contextlib.ExitStack()
        with ctx:
            keep = ctx.enter_context(tc.tile_pool(name="keep", bufs=1))
            work = ctx.enter_context(tc.tile_pool(name="work", bufs=2))

            act = nc.scalar.activation
            vec = nc.vector
            gps = nc.gpsimd

            # Ln bias consts: one written by each product engine so the Ln's
            # bias-read dependency merges with its u-input wait (single
            # foreign-engine wait per instruction).
            CBBs = []
            for c, eng in ((0, gps), (1, vec)):
                t = keep.tile([P, 1], f32, tag=f"CBB{c}", name=f"CBB{c}")
                eng.memset(t, CB)
                CBBs.append(t)

            def K(name, k, dt, w):
                return keep.tile([P, w * k], dt, tag=name, name=name)

            LLRs = [K(f"LLR{c}", 7, f32, WS[c]) for c in range(2)]
            LLHs = [K(f"LLH{c}", 7, f16, WS[c]) for c in range(2)]
            Ts   = [K(f"T{c}", 12, f32, WS[c]) for c in range(2)]
            Ms   = [K(f"M{c}", 12, f16, WS[c]) for c in range(2)]

            row0s = (0, P * WS[0])
            for c in range(2):
                W, LLR, LLH, T = WS[c], LLRs[c], LLHs[c], Ts[c]
                nc.sync.dma_start(out=LLR[:], in_=hbm_ap(llr_d, row0s[c], W))
                TL = keep.tile([P, W * 7], f32, tag=f"TL{c}", name=f"TL{c}")
                act(TL[:], LLR[:], F.Tanh, scale=0.5)
                # scatter tanh(llr/2) into the 12 edge slots (iteration-0 msgs)
                vec.tensor_copy(sub(T, 0, [[12, W], [1, 4]]),
                                sub(TL, 0, [[7, W], [2, 4]]))
                vec.tensor_copy(sub(T, 4, [[12, W], [1, 2]]),
                                sub(TL, 1, [[7, W], [1, 2]]))
                vec.tensor_copy(sub(T, 6, [[12, W], [1, 2]]),
                                sub(TL, 5, [[7, W], [1, 2]]))
                vec.tensor_copy(sub(T, 8, [[12, W], [1, 4]]),
                                sub(TL, 3, [[7, W], [1, 4]]))
                vec.tensor_copy(LLH[:], LLR[:])

            for it in range(iters):
                last = (it == iters - 1)
                for c in range(2):
                    W, LLR, LLH, T, M = WS[c], LLRs[c], LLHs[c], Ts[c], Ms[c]
                    prd = gps if c == 0 else vec   # product engine this chunk
                    CBB = CBBs[c]

                    def dyn9(t):
                        return sub(t, 1, [[12, W], [4, 3], [1, 3]])

                    def g12(t):
                        return sub(t, 0, [[12, W], [1, 12]])

                    def wt(name, k, dt):
                        return work.tile([P, W * k], dt, tag=f"{name}{c}",
                                         name=f"{name}{c}")

                    P6 = wt("P6", 6, f32)
                    U  = wt("U", 12, f32)
                    if last:
                        A  = wt("Af", 12, f32)
                        B  = wt("Bf", 12, f32)
                        CV = wt("CVf", 12, f32)
                        NL = wt("NLf", 7, f32)
                        LL = LLR
                    else:
                        A  = wt("Ah", 12, f16)
                        B  = wt("Bh", 12, f16)
                        CV = wt("CVh", 12, f16)
                        NL = wt("NLh", 7, f16)
                        LL = LLH

                    if it > 0:
                        act(dyn9(T), dyn9(M), F.Tanh, scale=0.5)

                    # pair products P6[2k+j] = t(slot 4k+2j) * t(slot 4k+2j+1)
                    prd.tensor_tensor(sub(P6, 0, [[6, W], [1, 6]]),
                                      sub(T, 0, [[12, W], [2, 6]]),
                                      sub(T, 1, [[12, W], [2, 6]]), Op.mult)
                    # leave-one-out: slots {2,3}: partner t * pair0 product
                    prd.tensor_tensor(sub(U, 2, [[12, W], [4, 3], [1, 2]]),
                                      sub(T, 3, [[12, W], [4, 3], [-1, 2]]),
                                      sub(P6, 0, [[6, W], [2, 3], [0, 2]]),
                                      Op.mult)
                    # slot {1}: static-partner t * pair1 product
                    prd.tensor_tensor(sub(U, 1, [[12, W], [4, 3]]),
                                      sub(T, 0, [[12, W], [4, 3]]),
                                      sub(P6, 1, [[6, W], [2, 3]]), Op.mult)
                    if last:
                        # static slots {0,4,8} (c2v for v0,v1,v3 outputs)
                        prd.tensor_tensor(sub(U, 0, [[12, W], [4, 3]]),
                                          sub(T, 1, [[12, W], [4, 3]]),
                                          sub(P6, 1, [[6, W], [2, 3]]), Op.mult)

                    sl = g12 if last else dyn9
                    act(sl(A), sl(U), F.Ln, bias=CBB[:], scale=CA)
                    act(sl(B), sl(U), F.Ln, bias=CBB[:], scale=-CA)
                    vec.tensor_tensor(sl(CV), sl(A), sl(B), Op.subtract)

                    # new_llr for feedback vars v2,v4,v5,v6
                    vec.tensor_tensor(sub(NL, 2, [[7, W], [2, 3]]),
                                      sub(LL, 2, [[7, W], [2, 3]]),
                                      sub(CV, 1, [[12, W], [1, 3]]), Op.add)
                    vec.tensor_tensor(sub(NL, 5, [[7, W], [1, 1]]),
                                      sub(LL, 5, [[7, W], [1, 1]]),
                                      sub(CV, 6, [[12, W], [1, 1]]), Op.add)
                    vec.tensor_tensor(sub(NL, 2, [[7, W], [4, 2]]),
                                      sub(NL, 2, [[7, W], [4, 2]]),
                                      sub(CV, 5, [[12, W], [2, 2]]), Op.add)
                    vec.tensor_tensor(sub(NL, 4, [[7, W], [1, 3]]),
                                      sub(NL, 4, [[7, W], [1, 3]]),
                                      sub(CV, 9, [[12, W], [1, 3]]), Op.add)

                    if last:
                        vec.tensor_tensor(sub(NL, 0, [[7, W], [1, 2]]),
                                          sub(LLR, 0, [[7, W], [1, 2]]),
                                          sub(CV, 0, [[12, W], [4, 2]]), Op.add)
                        vec.tensor_tensor(sub(NL, 3, [[7, W], [1, 1]]),
                                          sub(LLR, 3, [[7, W], [1, 1]]),
                                          sub(CV, 8, [[12, W], [1, 1]]), Op.add)
                        nc.sync.dma_start(out=hbm_ap(out_d, row0s[c], W),
                                          in_=NL[:])
                    else:
                        # m' = new_llr - c2v for the 9 dynamic edges
                        vec.tensor_tensor(sub(M, 1, [[12, W], [1, 3]]),
                                          sub(NL, 2, [[7, W], [2, 3]]),
                                          sub(CV, 1, [[12, W], [1, 3]]), Op.subtract)
                        vec.tensor_tensor(sub(M, 5, [[12, W], [1, 1]]),
                                          sub(NL, 2, [[7, W], [1, 1]]),
                                          sub(CV, 5, [[12, W], [1, 1]]), Op.subtract)
                        vec.tensor_tensor(sub(M, 6, [[12, W], [1, 2]]),
                                          sub(NL, 5, [[7, W], [1, 2]]),
                                          sub(CV, 6, [[12, W], [1, 2]]), Op.subtract)
                        vec.tensor_tensor(sub(M, 9, [[12, W], [1, 3]]),
                                          sub(NL, 4, [[7, W], [1, 3]]),
                                          sub(CV, 9, [[12, W], [1, 3]]), Op.subtract)

    # walrus on this stack supports a single sync-wait slot per instruction.
    # Tile emits (a) redundant same-engine waits (trivially satisfied by the
    # engine's FIFO program order once the preceding updates have happened)
    # and (b) a kernel-tail SP drain waiting on the whole global clock, where
    # only the output-DMA wait is load-bearing (the per-engine drain + EVSEM
    # butterfly that follows enforces engine completion).  Strip both.
    import bass_rust
    pref = {"EngineType.DVE": "DVE_", "EngineType.Pool": "Pool_",
            "EngineType.Activation": "Activation_", "EngineType.PE": "PE_",
            "EngineType.SP": "SP_"}
    inc = {}
    for b in nc.m.functions[0].blocks:
        for i in b.instructions:
            si = i.sync_info
            if si is None:
                continue
            if len(si.on_wait) > 1:
                if type(i).__name__ == "InstDrain":
                    dma = [w for w in si.on_wait if "DMA" in w.ant_name]
                    keep_w = dma[-1:] if dma else list(si.on_wait)[:1]
                else:
                    p = pref.get(str(i.engine))
                    keep_w = [w for w in si.on_wait
                              if not (p and w.ant_name.startswith(p)
                                      and w.wait_value <= inc.get(w.ant_name, 0))]
                    assert len(keep_w) <= 1, (i.name, [(w.ant_name, w.wait_value) for w in keep_w], {k: inc.get(k) for k in [w.ant_name for w in si.on_wait]})
                i.sync_info = bass_rust.SyncInfo(on_wait=keep_w,
                                                on_update=list(si.on_update))
                si = i.sync_info
            for u in si.on_update:
                if u.update_mode == "sem-inc":
                    inc[u.ant_name] = inc.get(u.ant_name, 0) + u.update_value
    return nc


def kernel(llr, max_iters):
    llr = np.ascontiguousarray(np.asarray(llr), dtype=np.float32)
    iters = int(np.asarray(max_iters))
    B = llr.shape[0]
    if iters <= 0:
        return llr.reshape(B, 1, 7).copy()

    from concourse.bass_utils import run_bass_kernel_spmd

    Bc = B // NCORES
    key = (Bc, iters)
    if key not in _CACHE:
        _CACHE[key] = _build(Bc, iters)
    nc = _CACHE[key]

    flat = llr.reshape(B, 7)
    in_maps = [{"llr": flat[i * Bc:(i + 1) * Bc]} for i in range(NCORES)]
    res = run_bass_kernel_spmd(nc, in_maps, core_ids=list(range(NCORES)))
    out = np.concatenate([np.asarray(r["out"]) for r in res.results], axis=0)
    return out.reshape(B, 1, 7)


# revision 9
# speedup vs baseline: 1.9441x; 1.4099x over previous
"""LDPC belief-propagation kernel for Trainium2 (8 NeuronCores, data-parallel).

Tanh-product formulation (per row; H fixed [3,7], 12 edges, check-major
slots with each check's degree-1 "static" edge in slot 0):
  t_e   = tanh(m_e / 2)                       (signed; ACT Tanh)
  u_e   = prod_{e' in check(e), e'!=e} t_e'   (leave-one-out via pair trick)
  c2v_e = ln(1+u_e) - ln(1-u_e)               (= 2 artanh(u); sign comes free)
  new_llr_v = llr_v + sum_{c ni v} c2v_{c,v}
  m'_e  = new_llr_v - c2v_e
This needs only 3 ACT ops/iter (Tanh + 2 Ln) vs 8 for the log-domain form.
Leave-one-out uses pair products: P(pair) = t_a*t_b, then
u_e = t_partner(e) * P(other pair).  Degree-1 variables (v0,v1,v3) have
constant messages == llr: their t values are computed once ("static" slots
0,4,8); per-iteration work covers only the 9 dynamic edges.  Iteration 0
messages equal llr_v, so its tanh is folded into the one-time setup.

Engine split: all transcendentals on ACT; the c2v/new_llr/m' post-path on
DVE (fp16 mid-iteration for the 2x_1p packed mode, fp32 on the last
iteration for the exact output path).  The t-products run on GPSIMD/Pool
for chunk 0 and on DVE for chunk 1 — Pool's 0.42 mult efficiency makes the
optimal batch split uneven, and dedicating one product engine per chunk
keeps every instruction dependent on at most one foreign engine (the
sync-strip pass below requires a single wait slot per instruction).

Slot layout (check-major):
  c0: [v0*, v2, v4, v6]  slots 0-3
  c1: [v1*, v2, v5, v6]  slots 4-7
  c2: [v3*, v4, v5, v6]  slots 8-11   (* = static, degree-1)
"""

import numpy as np

_CACHE = {}

NCORES = 8
P = 128                    # partitions
WS = (75, 74, 74, 33)      # free columns per partition per chunk (sum = Bc//P)
PRD_POOL = (True, True, True, False)  # product engine per chunk: Pool or DVE

CA = 0.99999988  # Ln scale so the argument stays >= 6e-8 even at u == +-1
CB = 0.99999994  # keeps c2v finite and |c2v| <= ~16.8 (matches ref clamp)


def _build(Bc, iters):
    import contextlib

    import concourse.bass as bass
    import concourse.tile as tile
    from concourse import mybir
    from concourse.alu_op_type import AluOpType as Op

    F = mybir.ActivationFunctionType
    assert Bc == P * sum(WS), (Bc, WS)
    f32 = mybir.dt.float32
    f16 = mybir.dt.float16

    nc = bass.Bass("TRN2", target_bir_lowering=False, debug=False,
                   num_devices=1)
    llr_d = nc.dram_tensor("llr", [Bc, 7], f32, kind="ExternalInput")
    out_d = nc.dram_tensor("out", [Bc, 7], f32, kind="ExternalOutput")

    def sub(t, off, dims):
        a = t[:] if callable(getattr(t, "__getitem__", None)) else t
        return bass.AP(tensor=a.tensor, offset=a.offset + off,
                       ap=[list(a.ap[0])] + [list(d) for d in dims])

    def hbm_ap(t, row0, w):
        # [P, 7w] view of rows [row0, row0 + P*w): partition p <-> w rows
        a = t.ap()
        return bass.AP(tensor=a.tensor, offset=a.offset + 7 * row0,
                       ap=[[7 * w, P], [1, 7 * w]])

    with tile.TileContext(nc) as tc:
        ctx = contextlib.ExitStack()
        with ctx:
            keep = ctx.enter_context(tc.tile_pool(name="keep", bufs=1))
            work = ctx.enter_context(tc.tile_pool(name="work", bufs=2))

            act = nc.scalar.activation
            vec = nc.vector
            gps = nc.gpsimd

            # Ln bias consts: one written by each product engine so the Ln's
            # bias-read dependency merges with its u-input wait (single
            # foreign-engine wait per instruction).
            CBBp = keep.tile([P, 1], f32, tag="CBBp", name="CBBp")
            gps.memset(CBBp, CB)
            CBBv = keep.tile([P, 1], f32, tag="CBBv", name="CBBv")
            vec.memset(CBBv, CB)
            CBBs = [CBBp if pp else CBBv for pp in PRD_POOL]

            def K(name, k, dt, w):
                return keep.tile([P, w * k], dt, tag=name, name=name)

            NCH = len(WS)
            LLRs = [K(f"LLR{c}", 7, f32, WS[c]) for c in range(NCH)]
            LLHs = [K(f"LLH{c}", 7, f16, WS[c]) for c in range(NCH)]
            Ts   = [K(f"T{c}", 12, f32, WS[c]) for c in range(NCH)]
            Ms   = [K(f"M{c}", 12, f16, WS[c]) for c in range(NCH)]

            row0s = [P * sum(WS[:c]) for c in range(NCH)]
            for c in range(NCH):
                W, LLR, LLH, T = WS[c], LLRs[c], LLHs[c], Ts[c]
                nc.sync.dma_start(out=LLR[:], in_=hbm_ap(llr_d, row0s[c], W))
                TL = keep.tile([P, W * 7], f32, tag=f"TL{c}", name=f"TL{c}")
                act(TL[:], LLR[:], F.Tanh, scale=0.5)
                # scatter tanh(llr/2) into the 12 edge slots (iteration-0 msgs)
                vec.tensor_copy(sub(T, 0, [[12, W], [1, 4]]),
                                sub(TL, 0, [[7, W], [2, 4]]))
                vec.tensor_copy(sub(T, 4, [[12, W], [1, 2]]),
                                sub(TL, 1, [[7, W], [1, 2]]))
                vec.tensor_copy(sub(T, 6, [[12, W], [1, 2]]),
                                sub(TL, 5, [[7, W], [1, 2]]))
                vec.tensor_copy(sub(T, 8, [[12, W], [1, 4]]),
                                sub(TL, 3, [[7, W], [1, 4]]))
                vec.tensor_copy(LLH[:], LLR[:])

            for it in range(iters):
                last = (it == iters - 1)
                for c in range(NCH):
                    W, LLR, LLH, T, M = WS[c], LLRs[c], LLHs[c], Ts[c], Ms[c]
                    prd = gps if PRD_POOL[c] else vec
                    CBB = CBBs[c]

                    def dyn9(t):
                        return sub(t, 1, [[12, W], [4, 3], [1, 3]])

                    def g12(t):
                        return sub(t, 0, [[12, W], [1, 12]])

                    def wt(name, k, dt):
                        return work.tile([P, W * k], dt, tag=f"{name}{c}",
                                         name=f"{name}{c}")

                    P6 = wt("P6", 6, f32)
                    U  = wt("U", 12, f32)
                    if last:
                        A  = wt("Af", 12, f32)
                        B  = wt("Bf", 12, f32)
                        CV = wt("CVf", 12, f32)
                        NL = wt("NLf", 7, f32)
                        LL = LLR
                    else:
                        A  = wt("Ah", 12, f16)
                        B  = wt("Bh", 12, f16)
                        CV = wt("CVh", 12, f16)
                        NL = wt("NLh", 7, f16)
                        LL = LLH

                    if it > 0:
                        act(dyn9(T), dyn9(M), F.Tanh, scale=0.5)

                    # pair products P6[2k+j] = t(slot 4k+2j) * t(slot 4k+2j+1)
                    prd.tensor_tensor(sub(P6, 0, [[6, W], [1, 6]]),
                                      sub(T, 0, [[12, W], [2, 6]]),
                                      sub(T, 1, [[12, W], [2, 6]]), Op.mult)
                    # leave-one-out: slots {2,3}: partner t * pair0 product
                    prd.tensor_tensor(sub(U, 2, [[12, W], [4, 3], [1, 2]]),
                                      sub(T, 3, [[12, W], [4, 3], [-1, 2]]),
                                      sub(P6, 0, [[6, W], [2, 3], [0, 2]]),
                                      Op.mult)
                    # slot {1}: static-partner t * pair1 product
                    prd.tensor_tensor(sub(U, 1, [[12, W], [4, 3]]),
                                      sub(T, 0, [[12, W], [4, 3]]),
                                      sub(P6, 1, [[6, W], [2, 3]]), Op.mult)
                    if last:
                        # static slots {0,4,8} (c2v for v0,v1,v3 outputs)
                        prd.tensor_tensor(sub(U, 0, [[12, W], [4, 3]]),
                                          sub(T, 1, [[12, W], [4, 3]]),
                                          sub(P6, 1, [[6, W], [2, 3]]), Op.mult)

                    sl = g12 if last else dyn9
                    act(sl(A), sl(U), F.Ln, bias=CBB[:], scale=CA)
                    act(sl(B), sl(U), F.Ln, bias=CBB[:], scale=-CA)
                    vec.tensor_tensor(sl(CV), sl(A), sl(B), Op.subtract)

                    # new_llr for feedback vars v2,v4,v5,v6
                    vec.tensor_tensor(sub(NL, 2, [[7, W], [2, 3]]),
                                      sub(LL, 2, [[7, W], [2, 3]]),
                                      sub(CV, 1, [[12, W], [1, 3]]), Op.add)
                    vec.tensor_tensor(sub(NL, 5, [[7, W], [1, 1]]),
                                      sub(LL, 5, [[7, W], [1, 1]]),
                                      sub(CV, 6, [[12, W], [1, 1]]), Op.add)
                    vec.tensor_tensor(sub(NL, 2, [[7, W], [4, 2]]),
                                      sub(NL, 2, [[7, W], [4, 2]]),
                                      sub(CV, 5, [[12, W], [2, 2]]), Op.add)
                    vec.tensor_tensor(sub(NL, 4, [[7, W], [1, 3]]),
                                      sub(NL, 4, [[7, W], [1, 3]]),
                                      sub(CV, 9, [[12, W], [1, 3]]), Op.add)

                    if last:
                        vec.tensor_tensor(sub(NL, 0, [[7, W], [1, 2]]),
                                          sub(LLR, 0, [[7, W], [1, 2]]),
                                          sub(CV, 0, [[12, W], [4, 2]]), Op.add)
                        vec.tensor_tensor(sub(NL, 3, [[7, W], [1, 1]]),
                                          sub(LLR, 3, [[7, W], [1, 1]]),
                                          sub(CV, 8, [[12, W], [1, 1]]), Op.add)
                        nc.sync.dma_start(out=hbm_ap(out_d, row0s[c], W),
                                          in_=NL[:])
                    else:
                        # m' = new_llr - c2v for the 9 dynamic edges
                        vec.tensor_tensor(sub(M, 1, [[12, W], [1, 3]]),
                                          sub(NL, 2, [[7, W], [2, 3]]),
                                          sub(CV, 1, [[12, W], [1, 3]]), Op.subtract)
                        vec.tensor_tensor(sub(M, 5, [[12, W], [1, 1]]),
                                          sub(NL, 2, [[7, W], [1, 1]]),
                                          sub(CV, 5, [[12, W], [1, 1]]), Op.subtract)
                        vec.tensor_tensor(sub(M, 6, [[12, W], [1, 2]]),
                                          sub(NL, 5, [[7, W], [1, 2]]),
                                          sub(CV, 6, [[12, W], [1, 2]]), Op.subtract)
                        vec.tensor_tensor(sub(M, 9, [[12, W], [1, 3]]),
                                          sub(NL, 4, [[7, W], [1, 3]]),
                                          sub(CV, 9, [[12, W], [1, 3]]), Op.subtract)

    # walrus on this stack supports a single sync-wait slot per instruction.
    # Tile emits (a) redundant same-engine waits (trivially satisfied by the
    # engine's FIFO program order once the preceding updates have happened)
    # and (b) a kernel-tail SP drain waiting on the whole global clock, where
    # only the output-DMA wait is load-bearing (the per-engine drain + EVSEM
    # butterfly that follows enforces engine completion).  Strip both.
    import bass_rust
    pref = {"EngineType.DVE": "DVE_", "EngineType.Pool": "Pool_",
            "EngineType.Activation": "Activation_", "EngineType.PE": "PE_",
            "EngineType.SP": "SP_"}
    inc = {}
    for b in nc.m.functions[0].blocks:
        for i in b.instructions:
            si = i.sync_info
            if si is None:
                continue
            if len(si.on_wait) > 1:
                if type(i).__name__ == "InstDrain":
                    dma = [w for w in si.on_wait if "DMA" in w.ant_name]
                    keep_w = dma[-1:] if dma else list(si.on_wait)[:1]
                else:
                    p = pref.get(str(i.engine))
                    keep_w = [w for w in si.on_wait
                              if not (p and w.ant_name.startswith(p)
                                      and w.wait_value <= inc.get(w.ant_name, 0))]
                    assert len(keep_w) <= 1, (i.name, [(w.ant_name, w.wait_value) for w in keep_w], {k: inc.get(k) for k in [w.ant_name for w in si.on_wait]})
                i.sync_info = bass_rust.SyncInfo(on_wait=keep_w,
                                                on_update=list(si.on_update))
                si = i.sync_info
            for u in si.on_update:
                if u.update_mode == "sem-inc":
                    inc[u.ant_name] = inc.get(u.ant_name, 0) + u.update_value
    return nc


def kernel(llr, max_iters):
    llr = np.ascontiguousarray(np.asarray(llr), dtype=np.float32)
    iters = int(np.asarray(max_iters))
    B = llr.shape[0]
    if iters <= 0:
        return llr.reshape(B, 1, 7).copy()

    from concourse.bass_utils import run_bass_kernel_spmd

    Bc = B // NCORES
    key = (Bc, iters)
    if key not in _CACHE:
        _CACHE[key] = _build(Bc, iters)
    nc = _CACHE[key]

    flat = llr.reshape(B, 7)
    in_maps = [{"llr": flat[i * Bc:(i + 1) * Bc]} for i in range(NCORES)]
    res = run_bass_kernel_spmd(nc, in_maps, core_ids=list(range(NCORES)))
    out = np.concatenate([np.asarray(r["out"]) for r in res.results], axis=0)
    return out.reshape(B, 1, 7)


# revision 10
# speedup vs baseline: 1.9738x; 1.0153x over previous
"""LDPC belief-propagation kernel for Trainium2 (8 NeuronCores, data-parallel).

Tanh-product formulation (per row; H fixed [3,7], 12 edges, check-major
slots with each check's degree-1 "static" edge in slot 0):
  t_e   = tanh(m_e / 2)                       (signed; ACT Tanh)
  u_e   = prod_{e' in check(e), e'!=e} t_e'   (leave-one-out via pair trick)
  c2v_e = ln(1+u_e) - ln(1-u_e)               (= 2 artanh(u); sign comes free)
  new_llr_v = llr_v + sum_{c ni v} c2v_{c,v}
  m'_e  = new_llr_v - c2v_e
This needs only 3 ACT ops/iter (Tanh + 2 Ln) vs 8 for the log-domain form.
Leave-one-out uses pair products: P(pair) = t_a*t_b, then
u_e = t_partner(e) * P(other pair).  Degree-1 variables (v0,v1,v3) have
constant messages == llr: their t values are computed once ("static" slots
0,4,8); per-iteration work covers only the 9 dynamic edges.  Iteration 0
messages equal llr_v, so its tanh is folded into the one-time setup.

Engine split: all transcendentals on ACT; the c2v/new_llr/m' post-path on
DVE (fp16 mid-iteration for the 2x_1p packed mode, fp32 on the last
iteration for the exact output path).  The t-products run on GPSIMD/Pool
for chunk 0 and on DVE for chunk 1 — Pool's 0.42 mult efficiency makes the
optimal batch split uneven, and dedicating one product engine per chunk
keeps every instruction dependent on at most one foreign engine (the
sync-strip pass below requires a single wait slot per instruction).

Slot layout (check-major):
  c0: [v0*, v2, v4, v6]  slots 0-3
  c1: [v1*, v2, v5, v6]  slots 4-7
  c2: [v3*, v4, v5, v6]  slots 8-11   (* = static, degree-1)
"""

import numpy as np

_CACHE = {}

NCORES = 8
P = 128                    # partitions
WS = (75, 74, 74, 33)      # free columns per partition per chunk (sum = Bc//P)
PRD_POOL = (True, True, True, False)  # product engine per chunk: Pool or DVE

CA = 0.99999988  # Ln scale so the argument stays >= 6e-8 even at u == +-1
CB = 0.99999994  # keeps c2v finite and |c2v| <= ~16.8 (matches ref clamp)


def _build(Bc, iters):
    import contextlib

    import concourse.bass as bass
    import concourse.tile as tile
    from concourse import mybir
    from concourse.alu_op_type import AluOpType as Op

    F = mybir.ActivationFunctionType
    assert Bc == P * sum(WS), (Bc, WS)
    f32 = mybir.dt.float32
    f16 = mybir.dt.float16

    nc = bass.Bass("TRN2", target_bir_lowering=False, debug=False,
                   num_devices=1)
    llr_d = nc.dram_tensor("llr", [Bc, 7], f32, kind="ExternalInput")
    out_d = nc.dram_tensor("out", [Bc, 7], f32, kind="ExternalOutput")

    def sub(t, off, dims):
        a = t[:] if callable(getattr(t, "__getitem__", None)) else t
        return bass.AP(tensor=a.tensor, offset=a.offset + off,
                       ap=[list(a.ap[0])] + [list(d) for d in dims])

    def hbm_ap(t, row0, w):
        # [P, 7w] view of rows [row0, row0 + P*w): partition p <-> w rows
        a = t.ap()
        return bass.AP(tensor=a.tensor, offset=a.offset + 7 * row0,
                       ap=[[7 * w, P], [1, 7 * w]])

    with tile.TileContext(nc) as tc:
        ctx = contextlib.ExitStack()
        with ctx:
            keep = ctx.enter_context(tc.tile_pool(name="keep", bufs=1))
            work = ctx.enter_context(tc.tile_pool(name="work", bufs=2))

            act = nc.scalar.activation
            vec = nc.vector
            gps = nc.gpsimd

            # Ln bias consts: one written by each product engine so the Ln's
            # bias-read dependency merges with its u-input wait (single
            # foreign-engine wait per instruction).
            CBBp = keep.tile([P, 1], f32, tag="CBBp", name="CBBp")
            gps.memset(CBBp, CB)
            CBBv = keep.tile([P, 1], f32, tag="CBBv", name="CBBv")
            vec.memset(CBBv, CB)
            CBBs = [CBBp if pp else CBBv for pp in PRD_POOL]

            def K(name, k, dt, w):
                return keep.tile([P, w * k], dt, tag=name, name=name)

            NCH = len(WS)
            LLRs = [K(f"LLR{c}", 7, f32, WS[c]) for c in range(NCH)]
            LLHs = [K(f"LLH{c}", 7, f16, WS[c]) for c in range(NCH)]
            Ts   = [K(f"T{c}", 12, f32, WS[c]) for c in range(NCH)]
            Ms   = [K(f"M{c}", 12, f16, WS[c]) for c in range(NCH)]

            row0s = [P * sum(WS[:c]) for c in range(NCH)]
            for c in range(NCH):
                W, LLR, LLH, T = WS[c], LLRs[c], LLHs[c], Ts[c]
                nc.sync.dma_start(out=LLR[:], in_=hbm_ap(llr_d, row0s[c], W))
                TL = keep.tile([P, W * 7], f32, tag=f"TL{c}", name=f"TL{c}")
                act(TL[:], LLR[:], F.Tanh, scale=0.5)
                # scatter tanh(llr/2) into the 12 edge slots (iteration-0 msgs)
                vec.tensor_copy(sub(T, 0, [[12, W], [1, 4]]),
                                sub(TL, 0, [[7, W], [2, 4]]))
                vec.tensor_copy(sub(T, 4, [[12, W], [1, 2]]),
                                sub(TL, 1, [[7, W], [1, 2]]))
                vec.tensor_copy(sub(T, 6, [[12, W], [1, 2]]),
                                sub(TL, 5, [[7, W], [1, 2]]))
                vec.tensor_copy(sub(T, 8, [[12, W], [1, 4]]),
                                sub(TL, 3, [[7, W], [1, 4]]))
                vec.tensor_copy(LLH[:], LLR[:])

            for it in range(iters):
                last = (it == iters - 1)

                def dyn9(t, W):
                    return sub(t, 1, [[12, W], [4, 3], [1, 3]])

                def g12(t, W):
                    return sub(t, 0, [[12, W], [1, 12]])

                def wt(name, c, k, dt):
                    return work.tile([P, WS[c] * k], dt, tag=f"{name}{c}",
                                     name=f"{name}{c}")

                # stage-major emission keeps every engine queue in dataflow
                # order across chunks (no head-of-line blocking)
                if it > 0:
                    for c in range(NCH):
                        act(dyn9(Ts[c], WS[c]), dyn9(Ms[c], WS[c]),
                            F.Tanh, scale=0.5)

                P6s, Us = [], []
                for c in range(NCH):
                    W, T = WS[c], Ts[c]
                    prd = gps if PRD_POOL[c] else vec
                    P6 = wt("P6", c, 6, f32)
                    U = wt("U", c, 12, f32)
                    P6s.append(P6); Us.append(U)
                    # pair products P6[2k+j] = t(4k+2j) * t(4k+2j+1)
                    prd.tensor_tensor(sub(P6, 0, [[6, W], [1, 6]]),
                                      sub(T, 0, [[12, W], [2, 6]]),
                                      sub(T, 1, [[12, W], [2, 6]]), Op.mult)
                    # leave-one-out slots {2,3}: partner t * pair0 product
                    prd.tensor_tensor(sub(U, 2, [[12, W], [4, 3], [1, 2]]),
                                      sub(T, 3, [[12, W], [4, 3], [-1, 2]]),
                                      sub(P6, 0, [[6, W], [2, 3], [0, 2]]),
                                      Op.mult)
                    # slot {1}: static-partner t * pair1 product
                    prd.tensor_tensor(sub(U, 1, [[12, W], [4, 3]]),
                                      sub(T, 0, [[12, W], [4, 3]]),
                                      sub(P6, 1, [[6, W], [2, 3]]), Op.mult)
                    if last:
                        # static slots {0,4,8} (c2v for v0,v1,v3 outputs)
                        prd.tensor_tensor(sub(U, 0, [[12, W], [4, 3]]),
                                          sub(T, 1, [[12, W], [4, 3]]),
                                          sub(P6, 1, [[6, W], [2, 3]]),
                                          Op.mult)

                ABs = []
                for c in range(NCH):
                    W, U = WS[c], Us[c]
                    dt = f32 if last else f16
                    A = wt("Af" if last else "Ah", c, 12, dt)
                    B = wt("Bf" if last else "Bh", c, 12, dt)
                    ABs.append((A, B))
                    sl = (lambda t: g12(t, W)) if last else (lambda t: dyn9(t, W))
                    act(sl(A), sl(U), F.Ln, bias=CBBs[c][:], scale=CA)
                    act(sl(B), sl(U), F.Ln, bias=CBBs[c][:], scale=-CA)

                for c in range(NCH):
                    W, LLR, LLH, M = WS[c], LLRs[c], LLHs[c], Ms[c]
                    A, B = ABs[c]
                    if last:
                        CV = wt("CVf", c, 12, f32)
                        NL = wt("NLf", c, 7, f32)
                        LL = LLR
                    else:
                        CV = wt("CVh", c, 12, f16)
                        NL = wt("NLh", c, 7, f16)
                        LL = LLH
                    sl = (lambda t: g12(t, W)) if last else (lambda t: dyn9(t, W))
                    vec.tensor_tensor(sl(CV), sl(A), sl(B), Op.subtract)

                    # new_llr for feedback vars v2,v4,v5,v6
                    vec.tensor_tensor(sub(NL, 2, [[7, W], [2, 3]]),
                                      sub(LL, 2, [[7, W], [2, 3]]),
                                      sub(CV, 1, [[12, W], [1, 3]]), Op.add)
                    vec.tensor_tensor(sub(NL, 5, [[7, W], [1, 1]]),
                                      sub(LL, 5, [[7, W], [1, 1]]),
                                      sub(CV, 6, [[12, W], [1, 1]]), Op.add)
                    vec.tensor_tensor(sub(NL, 2, [[7, W], [4, 2]]),
                                      sub(NL, 2, [[7, W], [4, 2]]),
                                      sub(CV, 5, [[12, W], [2, 2]]), Op.add)
                    vec.tensor_tensor(sub(NL, 4, [[7, W], [1, 3]]),
                                      sub(NL, 4, [[7, W], [1, 3]]),
                                      sub(CV, 9, [[12, W], [1, 3]]), Op.add)

                    if last:
                        vec.tensor_tensor(sub(NL, 0, [[7, W], [1, 2]]),
                                          sub(LLR, 0, [[7, W], [1, 2]]),
                                          sub(CV, 0, [[12, W], [4, 2]]), Op.add)
                        vec.tensor_tensor(sub(NL, 3, [[7, W], [1, 1]]),
                                          sub(LLR, 3, [[7, W], [1, 1]]),
                                          sub(CV, 8, [[12, W], [1, 1]]), Op.add)
                        nc.sync.dma_start(out=hbm_ap(out_d, row0s[c], W),
                                          in_=NL[:])
                    else:
                        # m' = new_llr - c2v for the 9 dynamic edges
                        vec.tensor_tensor(sub(M, 1, [[12, W], [1, 3]]),
                                          sub(NL, 2, [[7, W], [2, 3]]),
                                          sub(CV, 1, [[12, W], [1, 3]]),
                                          Op.subtract)
                        vec.tensor_tensor(sub(M, 5, [[12, W], [1, 1]]),
                                          sub(NL, 2, [[7, W], [1, 1]]),
                                          sub(CV, 5, [[12, W], [1, 1]]),
                                          Op.subtract)
                        vec.tensor_tensor(sub(M, 6, [[12, W], [1, 2]]),
                                          sub(NL, 5, [[7, W], [1, 2]]),
                                          sub(CV, 6, [[12, W], [1, 2]]),
                                          Op.subtract)
                        vec.tensor_tensor(sub(M, 9, [[12, W], [1, 3]]),
                                          sub(NL, 4, [[7, W], [1, 3]]),
                                          sub(CV, 9, [[12, W], [1, 3]]),
                                          Op.subtract)

    # walrus on this stack supports a single sync-wait slot per instruction.
    # Tile emits (a) redundant same-engine waits (trivially satisfied by the
    # engine's FIFO program order once the preceding updates have happened)
    # and (b) a kernel-tail SP drain waiting on the whole global clock, where
    # only the output-DMA wait is load-bearing (the per-engine drain + EVSEM
    # butterfly that follows enforces engine completion).  Strip both.
    import bass_rust
    pref = {"EngineType.DVE": "DVE_", "EngineType.Pool": "Pool_",
            "EngineType.Activation": "Activation_", "EngineType.PE": "PE_",
            "EngineType.SP": "SP_"}
    inc = {}
    for b in nc.m.functions[0].blocks:
        for i in b.instructions:
            si = i.sync_info
            if si is None:
                continue
            if len(si.on_wait) > 1:
                if type(i).__name__ == "InstDrain":
                    dma = [w for w in si.on_wait if "DMA" in w.ant_name]
                    keep_w = dma[-1:] if dma else list(si.on_wait)[:1]
                else:
                    p = pref.get(str(i.engine))
                    keep_w = [w for w in si.on_wait
                              if not (p and w.ant_name.startswith(p)
                                      and w.wait_value <= inc.get(w.ant_name, 0))]
                    assert len(keep_w) <= 1, (i.name, [(w.ant_name, w.wait_value) for w in keep_w], {k: inc.get(k) for k in [w.ant_name for w in si.on_wait]})
                i.sync_info = bass_rust.SyncInfo(on_wait=keep_w,
                                                on_update=list(si.on_update))
                si = i.sync_info
            for u in si.on_update:
                if u.update_mode == "sem-inc":
                    inc[u.ant_name] = inc.get(u.ant_name, 0) + u.update_value
    return nc


def kernel(llr, max_iters):
    llr = np.ascontiguousarray(np.asarray(llr), dtype=np.float32)
    iters = int(np.asarray(max_iters))
    B = llr.shape[0]
    if iters <= 0:
        return llr.reshape(B, 1, 7).copy()

    from concourse.bass_utils import run_bass_kernel_spmd

    Bc = B // NCORES
    key = (Bc, iters)
    if key not in _CACHE:
        _CACHE[key] = _build(Bc, iters)
    nc = _CACHE[key]

    flat = llr.reshape(B, 7)
    in_maps = [{"llr": flat[i * Bc:(i + 1) * Bc]} for i in range(NCORES)]
    res = run_bass_kernel_spmd(nc, in_maps, core_ids=list(range(NCORES)))
    out = np.concatenate([np.asarray(r["out"]) for r in res.results], axis=0)
    return out.reshape(B, 1, 7)
